# revision 1
# baseline (speedup 1.0000x reference)
"""Trainium2 Bass kernel for the MetricLearning pairwise loss.

Reference math:
    d2[i,j] = max(||x_i||^2 + ||x_j||^2 - 2 x_i.x_j, EPS)
    a = d2/(2k)/sigma^2 ; b = d2/(2k)/omega^2 ; c1 = k/2-1
    per_pair = same ? (-c1*log(a) + a/2) : (c1*log(b) - b/2)
    loss = sum_{i<j} per_pair

Per element, with L = log(d2) and t = x_i.x_j - sq_j/2 (so d2 = -2t + sq_i):
    diff_val = c1*L + B*t + c1*bias_q(i),  bias_q = logB - (B/(2c1))*sq_i
    same-diff correction = -2c1*L - (A+B)*t + c1*bias_w(i),
                           bias_w = -logA - logB + ((A+B)/(2c1))*sq_i
    loss = c1*SUM(L) + B*SUM(t)                      [over all pairs]
         - 2c1*SUM_same(L) - (A+B)*SUM_same(t)       [over same-label pairs]
         + c1*(sum_i bias_q(i)*cnt_main(i) + bias_w(i)*cnt_same(i))  [host]

Rows are globally SORTED BY LABEL, so same-label pairs live only within a
block or in the corner between consecutive blocks (label runs < 128 rows).
The main term therefore needs NO label mask at all (ACT accум + one DVE
reduce per tile); the correction runs on 6 small regions per core.

Sharding: 16 row-blocks of 256; the K16 block-pair graph is oriented so
every core owns one even block (8 partners) + one odd block (7 partners)
plus both within-block triangles -> identical SPMD program on all 8 cores,
per-core variation only in input data (slab permutation).
"""

import numpy as np
import ml_dtypes

N = 4096
D = 1024
P = 128
NB = 16          # row blocks
BLK = 256        # rows per block
KC = D // P      # k chunks (8)
NCORES = 8

SIGMA = 0.2
OMEGA = 1.0
K_F = float(N)
C1 = K_F / 2.0 - 1.0                      # 2047
A_C = 1.0 / (2.0 * K_F * SIGMA * SIGMA)   # 1/327.68
B_C = 1.0 / (2.0 * K_F * OMEGA * OMEGA)   # 1/8192
LOG_A = float(np.log(A_C))
LOG_B = float(np.log(B_C))
EPS_D2 = 1e-3   # clamp floor for the (masked-out) diagonal; real d2 >= ~1500

# job := (lhs_slab in {0,1}, unit u in {0,1}, col_lo in slots*BLK, width, diag)
JOBS = []
for _u in (0, 1):
    JOBS.append((0, _u, 0, 256, True))
    JOBS.append((1, _u, 256, 256, True))
for _u in (0, 1):
    for _g in ((256, 512), (768, 512), (1280, 512), (1792, 512)):
        JOBS.append((0, _u, _g[0], _g[1], False))
    for _g in ((2304, 512), (2816, 512), (3328, 512), (3840, 256)):
        JOBS.append((1, _u, _g[0], _g[1], False))
NJOBS = len(JOBS)  # 20

# correction regions: (job_idx, corner?) — diag jobs (0..3) get in-tile
# upper-triangle same-label correction; the two u=1 cross jobs that start
# at slot1 / slot9 get a 128-wide corner correction (consecutive blocks).
DIAG_JOBS = [ji for ji, j in enumerate(JOBS) if j[4]]
CORNER_JOBS = [ji for ji, j in enumerate(JOBS)
               if not j[4] and j[1] == 1 and j[2] in (256, 2304)]
CORNER_W = 128

# acc column map (raw sums; coefficients applied in the final dot).
# (L,T) sums sit in adjacent even/odd columns so a single [P,2,w] reduce can
# write both; same for (ML,MT).
ACC_W = 64
COL_L = {ji: 2 * ji for ji in range(NJOBS)}          # even 0..38   coeff c1
COL_T = {ji: 2 * ji + 1 for ji in range(NJOBS)}      # odd  1..39   coeff B
_corr = DIAG_JOBS + CORNER_JOBS
COL_ML = {ji: 40 + 2 * k for k, ji in enumerate(_corr)}      # coeff -2c1
COL_MT = {ji: 41 + 2 * k for k, ji in enumerate(_corr)}      # coeff -(A+B)
COEFS = [C1, B_C, -2.0 * C1, -(A_C + B_C)]
# groups as (start, step, count) over acc columns
GROUPS = [(0, 2, 20), (1, 2, 20), (40, 2, 6), (41, 2, 6)]

# emission order: DMA-arrival aligned, diag (heavy-epilogue) jobs interleaved
JOB_ORDER = [0, 2, 1, 3, 4, 12, 5, 13, 6, 14, 7, 15, 8, 16, 9, 17, 10, 18,
             11, 19]


def _partners(d):
    """Block orientation: edge {i,j} (i<j) owned by i if i+j odd else j."""
    l0, l1 = 2 * d, 2 * d + 1
    p8 = [j for j in range(l0 + 1, NB) if j % 2 == 1] + \
         [i for i in range(0, l0) if i % 2 == 0]
    p7 = [j for j in range(l1 + 1, NB) if j % 2 == 0] + \
         [i for i in range(0, l1) if i % 2 == 1]
    assert len(p8) == 8 and len(p7) == 7 and l1 in p8
    return l0, l1, p8, p7


def _core_slabs(d):
    """Slot -> block id (16 slots). slot0=own even, slot1=own odd, and
    slot9 (first partner of the odd block) pinned to block 2d+2 when it
    exists so the consecutive-pair corner lands at a fixed slot."""
    l0, l1, p8, p7 = _partners(d)
    rest8 = [p for p in p8 if p != l1]
    nxt = l1 + 1
    if nxt in p7:
        p7 = [nxt] + [p for p in p7 if p != nxt]
    slabs = [l0, l1] + rest8 + list(p7)
    assert len(slabs) == NB and len(set(slabs)) == NB
    return slabs


_PROG_CACHE = {}


def _build_program():
    if "nc" in _PROG_CACHE:
        return _PROG_CACHE["nc"]
    import concourse.bass as bass  # noqa: F401
    import concourse.bacc as bacc
    import concourse.mybir as mybir
    import concourse.tile as tile

    F32 = mybir.dt.float32
    BF16 = mybir.dt.bfloat16
    FP8 = mybir.dt.float8e4
    AF = mybir.ActivationFunctionType
    ALU = mybir.AluOpType

    nc = bacc.Bacc("TRN2", target_bir_lowering=False, debug=False,
                   num_devices=NCORES)
    xtp_d = nc.dram_tensor("xtp", [NB, P, KC, BLK], FP8,
                           kind="ExternalInput").ap()
    aug_d = nc.dram_tensor("aug", [2, N], BF16, kind="ExternalInput").ap()
    lab_d = nc.dram_tensor("lab", [1, 640], BF16, kind="ExternalInput").ap()
    rowd_d = nc.dram_tensor("rowd", [P, 4 * 3], F32, kind="ExternalInput").ap()
    coef_d = nc.dram_tensor("coef", [4, 1], F32, kind="ExternalInput").ap()
    out_d = nc.dram_tensor("out", [1, 1], F32, kind="ExternalOutput").ap()

    with tile.TileContext(nc) as tc:
        with (
            tc.tile_pool(name="persist", bufs=1) as persist,
            tc.tile_pool(name="scratch", bufs=3) as scratch,
            tc.tile_pool(name="dscratch", bufs=2) as dscratch,
            tc.tile_pool(name="psum", bufs=6, space="PSUM") as psum,
            tc.tile_pool(name="psumloss", bufs=1, space="PSUM") as psumloss,
        ):
            xall = persist.tile([P, KC, NB, BLK], FP8, tag="xall")
            labb = persist.tile([P, 640], F32, tag="labb")
            labr = persist.tile([1, 640], BF16, tag="labr")
            augs = persist.tile([2, N], BF16, tag="augs")
            rd = persist.tile([P, 4 * 3], F32, tag="rd")
            coefs = persist.tile([4, 1], F32, tag="coefs")
            ones2 = persist.tile([2, P], BF16, tag="ones2")
            ones1f = persist.tile([P, 1], F32, tag="ones1f")
            acc = persist.tile([P, ACC_W], F32, tag="acc")
            g4 = persist.tile([P, 4], F32, tag="g4")
            c4 = persist.tile([4, 1], F32, tag="c4")
            lossb = persist.tile([1, 1], F32, tag="lossb")

            def load_slab(s):
                nc.sync.dma_start(out=xall[:, :, s, :], in_=xtp_d[s])
            nc.scalar.dma_start(out=labr[:], in_=lab_d[:])
            nc.scalar.dma_start(out=augs[:], in_=aug_d[:])
            nc.scalar.dma_start(out=rd[:], in_=rowd_d[:])
            nc.scalar.dma_start(out=coefs[:], in_=coef_d[:])
            for s in range(NB):
                load_slab(s)

            nc.gpsimd.memset(ones2[:], 1.0)
            nc.gpsimd.memset(ones1f[:], 1.0)
            nc.gpsimd.memset(acc[:], 0.0)

            def lab_bcast():
                # broadcast the 640-wide label row across partitions via PE
                for lo, w in ((0, 512), (512, 128)):
                    pl = psum.tile([P, w], F32, tag="gram")
                    nc.tensor.matmul(pl[:], ones2[0:1, :],
                                     labr[0:1, lo:lo + w],
                                     start=True, stop=True)
                    nc.vector.tensor_copy(labb[:, lo:lo + w], pl[:])

            lab_bcast()
            for oi, ji in enumerate(JOB_ORDER):
                ls, u, clo, wid, diag = JOBS[ji]
                g = 2 * ls + u
                sq_ap = rd[:, 3 * g + 0:3 * g + 1]
                lb_ap = rd[:, 3 * g + 1:3 * g + 2]
                th_ap = rd[:, 3 * g + 2:3 * g + 3]

                t = psum.tile([P, wid], F32, tag="gram")
                ns = wid // BLK
                s0 = clo // BLK
                for kc2 in range(KC // 2):
                    nc.tensor.matmul(
                        t[:],
                        xall[:, 2 * kc2:2 * kc2 + 2, ls,
                             128 * u:128 * (u + 1)],
                        xall[:, 2 * kc2:2 * kc2 + 2, s0:s0 + ns, :],
                        start=(kc2 == 0), stop=False,
                        perf_mode=mybir.MatmulPerfMode.DoubleRow,
                    )
                nc.tensor.matmul(t[:], ones2[:, :],
                                 augs[:, clo:clo + wid],
                                 start=False, stop=True)

                if not diag:
                    Lt = scratch.tile([P, wid], F32, tag="L")
                    nc.scalar.activation(Lt[:], t[:], AF.Ln,
                                         bias=sq_ap, scale=-2.0,
                                         accum_out=acc[:, COL_L[ji]:
                                                       COL_L[ji] + 1])
                    nc.vector.tensor_reduce(
                        acc[:, COL_T[ji]:COL_T[ji] + 1], t[:],
                        axis=mybir.AxisListType.X, op=ALU.add)
                    if ji in CORNER_JOBS:
                        # same-label corner vs the consecutive block
                        labwin = labb[:, 256:384] if clo == 256 else \
                            labb[:, 512:640]
                        cw = CORNER_W
                        m = dscratch.tile([P, cw], F32, tag="mc")
                        nc.vector.tensor_scalar(m[:], labwin, lb_ap, None,
                                                ALU.is_equal)
                        prod = dscratch.tile([P, 2 * cw], F32, tag="pc")
                        nc.vector.tensor_tensor(prod[:, 0:cw], m[:],
                                                Lt[:, 0:cw], ALU.mult)
                        nc.vector.tensor_tensor(prod[:, cw:2 * cw], m[:],
                                                t[:, 0:cw], ALU.mult)
                        nc.vector.tensor_reduce(
                            acc[:, COL_ML[ji]:COL_ML[ji] + 2],
                            prod[:].rearrange("p (two w) -> p two w", two=2),
                            axis=mybir.AxisListType.X, op=ALU.add)
                else:
                    # clamp (protects the exact diagonal), log, strict-upper
                    t2 = dscratch.tile([P, wid], F32, tag="t2")
                    nc.vector.tensor_scalar(t2[:], t[:], th_ap, None, ALU.min)
                    Lt = scratch.tile([P, wid], F32, tag="L")
                    nc.scalar.activation(Lt[:], t2[:], AF.Ln,
                                         bias=sq_ap, scale=-2.0)
                    up = dscratch.tile([P, 2 * wid], F32, tag="up")
                    for src, off in ((Lt, 0), (t2, wid)):
                        nc.gpsimd.affine_select(
                            out=up[:, off:off + wid], in_=src[:],
                            compare_op=ALU.is_gt, fill=0.0,
                            base=-128 * u, channel_multiplier=-1,
                            pattern=[[1, wid]],
                        )
                    nc.vector.tensor_reduce(
                        acc[:, COL_L[ji]:COL_L[ji] + 2],
                        up[:].rearrange("p (two w) -> p two w", two=2),
                        axis=mybir.AxisListType.X, op=ALU.add)
                    # same-label correction, strict upper only
                    labwin = labb[:, 256 * ls:256 * ls + wid]
                    m = dscratch.tile([P, wid], F32, tag="md")
                    nc.vector.tensor_scalar(m[:], labwin, lb_ap, None,
                                            ALU.is_equal)
                    mu = dscratch.tile([P, wid], F32, tag="mu")
                    nc.gpsimd.affine_select(
                        out=mu[:], in_=m[:], compare_op=ALU.is_gt, fill=0.0,
                        base=-128 * u, channel_multiplier=-1,
                        pattern=[[1, wid]],
                    )
                    # mu broadcast over the [L' | t2'] concat: one product
                    prod = dscratch.tile([P, 2 * wid], F32, tag="pd")
                    nc.vector.tensor_tensor(
                        prod[:].rearrange("p (two w) -> p two w", two=2),
                        mu[:].rearrange("p (one w) -> p one w", one=1)
                             .broadcast_to([P, 2, wid]),
                        up[:].rearrange("p (two w) -> p two w", two=2),
                        ALU.mult)
                    nc.vector.tensor_reduce(
                        acc[:, COL_ML[ji]:COL_ML[ji] + 2],
                        prod[:].rearrange("p (two w) -> p two w", two=2),
                        axis=mybir.AxisListType.X, op=ALU.add)

            # final: group-reduce acc, weight by coefs via two tiny matmuls
            for k, (lo, step, cnt) in enumerate(GROUPS):
                nc.vector.tensor_reduce(g4[:, k:k + 1],
                                        acc[:, lo:lo + step * cnt:step],
                                        axis=mybir.AxisListType.X, op=ALU.add)
            p4 = psumloss.tile([4, 1], F32, tag="fin")
            nc.tensor.matmul(p4[:], g4[:], ones1f[:], start=True, stop=True)
            nc.scalar.activation(c4[:], p4[:], AF.Copy)
            pls = psumloss.tile([1, 1], F32, tag="fin")
            nc.tensor.matmul(pls[:], c4[:], coefs[:], start=True, stop=True)
            nc.scalar.activation(lossb[:], pls[:], AF.Copy)
            nc.sync.dma_start(out=out_d[:], in_=lossb[:])

    nc.compile()
    _PROG_CACHE["nc"] = nc
    return nc


def _host_prep(outputs, labels):
    """Sort rows by label, build per-core inputs + the host constant."""
    x = np.asarray(outputs, dtype=np.float32)
    lab = np.asarray(labels)
    assert x.shape == (N, D)
    perm = np.argsort(lab, kind="stable")
    xp = x[perm]
    labp = lab[perm].astype(np.float64)

    # label runs (sorted) -> cnt_same(i) = run_end(i) - i - 1
    runs_end = np.empty(N, dtype=np.int64)
    i = 0
    max_run = 0
    while i < N:
        j = i
        while j < N and labp[j] == labp[i]:
            j += 1
        runs_end[i:j] = j
        max_run = max(max_run, j - i)
        i = j
    assert max_run <= CORNER_W, f"label run {max_run} exceeds corner width"
    cnt_same = runs_end - np.arange(N) - 1

    # cnt_main(i) = BLK*outdeg(block) + (BLK-1 - (i % BLK))
    blocks = np.arange(N) // BLK
    outdeg = np.where(blocks % 2 == 0, 8, 7)
    cnt_main = BLK * outdeg + (BLK - 1 - (np.arange(N) % BLK))

    xq = xp.astype(ml_dtypes.float8_e4m3)
    # True (unquantized) norms make d2 = sq_i + sq_j - 2*xq_i.xq_j unbiased:
    # the value-error correlation in ||xq||^2 cancels the ||e||^2 term.
    sq = (xp.astype(np.float64) ** 2).sum(axis=1)
    bias_q = LOG_B - (B_C / (2 * C1)) * sq
    bias_w = -LOG_A - LOG_B + ((A_C + B_C) / (2 * C1)) * sq
    host_add = C1 * float((bias_q * cnt_main).sum()
                          + (bias_w * cnt_same).sum())

    xt_q = np.ascontiguousarray(xq.T)                               # [D, N]
    neg_half = -0.5 * sq
    hi = neg_half.astype(ml_dtypes.bfloat16)
    lo = (neg_half - hi.astype(np.float64)).astype(ml_dtypes.bfloat16)

    coef = np.asarray(COEFS, dtype=np.float32).reshape(4, 1)

    in_maps = []
    for d in range(NCORES):
        slabs = _core_slabs(d)
        cols = np.concatenate(
            [np.arange(b * BLK, (b + 1) * BLK) for b in slabs])
        xtp = np.ascontiguousarray(
            xt_q[:, cols].reshape(KC, P, NB, BLK).transpose(2, 1, 0, 3))
        aug = np.stack([hi[cols], lo[cols]])                       # [2, N]
        # label row for slot0(256) | slot1(256) | slot9 first 128
        lcols = np.concatenate([cols[0:512], cols[9 * BLK:9 * BLK + 128]])
        labrow = labp[lcols].astype(ml_dtypes.bfloat16)[None, :]   # [1, 640]

        rowd = np.zeros((P, 4 * 3), dtype=np.float64)
        for g, (slab, u) in enumerate(((0, 0), (0, 1), (1, 0), (1, 1))):
            rows = slabs[slab] * BLK + 128 * u + np.arange(P)
            sqr = sq[rows]
            rowd[:, 3 * g + 0] = sqr
            rowd[:, 3 * g + 1] = labp[rows]
            rowd[:, 3 * g + 2] = (sqr - EPS_D2) / 2.0
        in_maps.append({
            "xtp": xtp,
            "aug": np.ascontiguousarray(aug),
            "lab": np.ascontiguousarray(labrow),
            "rowd": rowd.astype(np.float32),
            "coef": coef,
        })
    return in_maps, host_add


def kernel(**inputs):
    from concourse.bass_utils import run_bass_kernel_spmd
    nc = _build_program()
    in_maps, host_add = _host_prep(inputs["outputs"], inputs["labels"])
    res = run_bass_kernel_spmd(nc, in_maps, core_ids=list(range(NCORES)))
    total = np.float64(host_add)
    for r in res.results:
        total += np.float64(r["out"][0, 0])
    return np.asarray(total, dtype=np.float32)



# revision 2
# speedup vs baseline: 1.0196x; 1.0196x over previous
"""Trainium2 Bass kernel for the MetricLearning pairwise loss.

Reference math:
    d2[i,j] = max(||x_i||^2 + ||x_j||^2 - 2 x_i.x_j, EPS)
    a = d2/(2k)/sigma^2 ; b = d2/(2k)/omega^2 ; c1 = k/2-1
    per_pair = same ? (-c1*log(a) + a/2) : (c1*log(b) - b/2)
    loss = sum_{i<j} per_pair

Split per pair (L = log d2):
    diff formula on every pair:  c1*(L + lnB) - (B/2)*d2
    same-pair correction:        -2c1*L - c1*(lnA+lnB) + ((A+B)/2)*d2
All terms linear in d2 and the pair counts are computed on the HOST in
fp64 over the exact quantized data; only the log sums need the device:
    S1 = sum L over cross-block pairs (one orientation each)
    S2 = sum L over full 256x256 diagonal blocks (diag pinned to EPS_D2)
    S3 = sum same-mask * L over full diagonal blocks (bf16 L)
    S4 = sum same-mask * L over block-boundary corners (i<j rows x cols)
    loss = c1*S1 + (c1/2)*(S2 - N*lnE) - c1*(S3 - N*lnE_bf) - 2c1*S4 + host
Triangle masks are gone: within-block sums use the double-count identity
sum_{i<j} = (sum_full - sum_diag)/2 (mask is symmetric), and the diagonal
is clamped so that d2_ii == EPS_D2 exactly (fp32-exact subtraction).

Rows are globally SORTED BY LABEL (runs < 128 rows), so same-label pairs
live within a block or in the 128-wide corner between consecutive blocks.

Per core (SPMD, K16 edge orientation): lhs blocks l0=2d, l1=2d+1; 9 tiles
per 128-row unit u: A=[l0 diag|edge l0-l1], F=[l1 diag|edge l1-l1+1],
plus 7 pure-cross tiles. The aug matmul (adds -sq_j/2, K=2 bf16 hi/lo)
issues FIRST in each PSUM group so every LDWEIGHTS hides under the
previous matmul's stream. Bulk input arrives as 5 grouped DMAs with 1KB
descriptors on two HW queues; dummy fp8 matmuls + a dummy Ln warm the PE
clock (HAM) and the ACT table during the fill.
"""

import numpy as np
import ml_dtypes

N = 4096
D = 1024
P = 128
NB = 16          # row blocks
BLK = 256        # rows per block
KC = D // P      # k chunks (8)
NCORES = 8

SIGMA = 0.2
OMEGA = 1.0
K_F = float(N)
C1 = K_F / 2.0 - 1.0                      # 2047
A_C = 1.0 / (2.0 * K_F * SIGMA * SIGMA)
B_C = 1.0 / (2.0 * K_F * OMEGA * OMEGA)
LOG_A = float(np.log(A_C))
LOG_B = float(np.log(B_C))
EPS_D2 = 256.0   # diagonal pin value; real off-diag d2 >= ~1500
LNE = float(np.log(EPS_D2))
LNE_BF = float(ml_dtypes.bfloat16(np.log(EPS_D2)))

# tiles per unit: (lhs_ls in {0,1}, first col slot, n slots, kind)
TILES = [
    (0, 0, 2, "A"),   # l0 diag + edge (l0,l1)
    (1, 1, 2, "F"),   # l1 diag + edge (l1,corner)
    (0, 3, 2, "X"),
    (0, 5, 2, "X"),
    (0, 7, 2, "X"),
    (0, 9, 1, "X"),
    (1, 10, 2, "X"),
    (1, 12, 2, "X"),
    (1, 14, 2, "X"),
]
# emission order: tile-major, u inner (matches DMA group arrival)
PHASES = [(ti, u) for ti in range(len(TILES)) for u in (0, 1)]
NPH = len(PHASES)  # 18

# acc column map
COL_X = {}     # phase -> cross-L col (coeff c1)
COL_D = {}     # diag-L col (coeff c1/2)
COL_M = {}     # diag mask-L col (coeff -c1)
COL_C = {}     # corner mask-L col (coeff -2c1)
_c = 0
for _pi, (_ti, _u) in enumerate(PHASES):
    COL_X[_pi] = _c; _c += 1
for _pi, (_ti, _u) in enumerate(PHASES):
    if TILES[_ti][3] != "X":
        COL_D[_pi] = _c; _c += 1
for _pi, (_ti, _u) in enumerate(PHASES):
    if TILES[_ti][3] != "X":
        COL_M[_pi] = _c; _c += 1
for _pi, (_ti, _u) in enumerate(PHASES):
    if TILES[_ti][3] != "X" and _u == 1:
        COL_C[_pi] = _c; _c += 1
ACC_W = 32
assert _c <= ACC_W

# bulk DMA slab groups (slot ranges) and their queue assignment
GROUPS = [(0, 2), (2, 2), (4, 4), (8, 4), (12, 4)]
GROUP_Q = ["sync", "scalar", "sync", "scalar", "sync"]


def _partners(d):
    """Block orientation: edge {i,j} (i<j) owned by i if i+j odd else j."""
    l0, l1 = 2 * d, 2 * d + 1
    p8 = [j for j in range(l0 + 1, NB) if j % 2 == 1] + \
         [i for i in range(0, l0) if i % 2 == 0]
    p7 = [j for j in range(l1 + 1, NB) if j % 2 == 0] + \
         [i for i in range(0, l1) if i % 2 == 1]
    assert len(p8) == 8 and len(p7) == 7 and l1 in p8
    return l0, l1, p8, p7


def _core_slabs(d):
    """Slot -> block id. slot0=l0, slot1=l1, slot2=corner partner
    (l1+1 when it exists, so the consecutive-pair corner sits in tile F)."""
    l0, l1, p8, p7 = _partners(d)
    nxt = l1 + 1
    corner = nxt if nxt in p7 else p7[0]
    rest8 = [p for p in p8 if p != l1]
    rest7 = [p for p in p7 if p != corner]
    slabs = [l0, l1, corner] + rest8 + rest7
    assert len(slabs) == NB and len(set(slabs)) == NB
    return slabs


_PROG_CACHE = {}


def _build_program():
    if "nc" in _PROG_CACHE:
        return _PROG_CACHE["nc"]
    import concourse.bass as bass  # noqa: F401
    import concourse.bacc as bacc
    import concourse.mybir as mybir
    import concourse.tile as tile

    F32 = mybir.dt.float32
    BF16 = mybir.dt.bfloat16
    FP8 = mybir.dt.float8e4
    AF = mybir.ActivationFunctionType
    ALU = mybir.AluOpType
    DR = mybir.MatmulPerfMode.DoubleRow

    nc = bacc.Bacc("TRN2", target_bir_lowering=False, debug=False,
                   num_devices=NCORES)
    xg_d = [nc.dram_tensor(f"xg{i}", [P, KC, gs, BLK], FP8,
                           kind="ExternalInput").ap()
            for i, (_, gs) in enumerate(GROUPS)]
    aug_d = nc.dram_tensor("aug", [2, N], BF16, kind="ExternalInput").ap()
    lab_d = nc.dram_tensor("lab", [1, 640], BF16, kind="ExternalInput").ap()
    rowd_d = nc.dram_tensor("rowd", [P, 4 * 3], F32, kind="ExternalInput").ap()
    out_d = nc.dram_tensor("out", [P, ACC_W], F32, kind="ExternalOutput").ap()

    with tile.TileContext(nc) as tc:
        with (
            tc.tile_pool(name="persist", bufs=1) as persist,
            tc.tile_pool(name="lbuf", bufs=2) as lpool,
            tc.tile_pool(name="dscratch", bufs=2) as dscratch,
            tc.tile_pool(name="psum", bufs=4, space="PSUM") as psum,
            tc.tile_pool(name="psumw", bufs=1, space="PSUM") as psumw,
        ):
            xall = persist.tile([P, KC, NB, BLK], FP8, tag="xall")
            labb = persist.tile([P, 640], F32, tag="labb")
            labr = persist.tile([1, 640], BF16, tag="labr")
            augs = persist.tile([2, N], BF16, tag="augs")
            rd = persist.tile([P, 4 * 3], F32, tag="rd")
            ones2 = persist.tile([2, P], BF16, tag="ones2")
            junk = persist.tile([P, 2, 768], FP8, tag="junk")
            acc = persist.tile([P, ACC_W], F32, tag="acc")
            ldump = persist.tile([P, 512], BF16, tag="ldump")

            # small inputs first on the scalar queue
            nc.scalar.dma_start(out=labr[:], in_=lab_d[:])
            nc.scalar.dma_start(out=rd[:], in_=rowd_d[:])
            nc.scalar.dma_start(out=augs[:], in_=aug_d[:])

            # constants (gpsimd)
            nc.gpsimd.memset(ones2[:], 1.0)
            nc.gpsimd.memset(junk[:], 0.0)
            nc.gpsimd.memset(acc[:], 0.0)

            # bulk slab groups: big-descriptor DMAs on two HW queues
            for gi, (s0, gs) in enumerate(GROUPS):
                eng = nc.sync if GROUP_Q[gi] == "sync" else nc.scalar
                eng.dma_start(out=xall[:, :, s0:s0 + gs, :], in_=xg_d[gi])

            # PE warm-up (HAM) on junk data while DMAs fill SBUF
            wt = psumw.tile([P, 512], F32, tag="warm")
            for _ in range(5):
                nc.tensor.matmul(wt[:], junk[:, :, 0:128], junk[:, :, 128:640],
                                 start=True, stop=True, perf_mode=DR)
            # dummy Ln loads the ACT table set early
            nc.scalar.activation(ldump[0:2, 0:128], ones2[:], AF.Ln)

            # broadcast the 640-wide label row across partitions via PE
            for lo, w in ((0, 512), (512, 128)):
                pl = psumw.tile([P, w], F32, tag="warm")
                nc.tensor.matmul(pl[:], ones2[0:1, :], labr[0:1, lo:lo + w],
                                 start=True, stop=True)
                nc.vector.tensor_copy(labb[:, lo:lo + w], pl[:])

            for pi, (ti, u) in enumerate(PHASES):
                ls, slot0, ns, kind = TILES[ti]
                wid = ns * BLK
                clo = slot0 * BLK
                g = 2 * ls + u
                sq_ap = rd[:, 3 * g + 0:3 * g + 1]
                lb_ap = rd[:, 3 * g + 1:3 * g + 2]
                th_ap = rd[:, 3 * g + 2:3 * g + 3]

                t = psum.tile([P, wid], F32, tag="gram")
                # aug first: every LDWEIGHTS hides under a matmul stream
                nc.tensor.matmul(t[:], ones2[:, :], augs[:, clo:clo + wid],
                                 start=True, stop=False)
                for kc2 in range(KC // 2):
                    nc.tensor.matmul(
                        t[:],
                        xall[:, 2 * kc2:2 * kc2 + 2, ls, 128 * u:128 * (u + 1)],
                        xall[:, 2 * kc2:2 * kc2 + 2, slot0:slot0 + ns, :],
                        start=False, stop=(kc2 == KC // 2 - 1),
                        perf_mode=DR,
                    )

                if kind == "X":
                    nc.scalar.activation(ldump[:, 0:wid], t[:], AF.Ln,
                                         bias=sq_ap, scale=-2.0,
                                         accum_out=acc[:, COL_X[pi]:
                                                       COL_X[pi] + 1])
                else:
                    # diag half: pin the diagonal to d2 == EPS_D2 via min
                    t2 = dscratch.tile([P, 256], F32, tag="t2")
                    nc.vector.tensor_scalar(t2[:], t[:, 0:256], th_ap, None,
                                            ALU.min)
                    lb = lpool.tile([P, 512], BF16, tag="L")
                    nc.scalar.activation(lb[:, 0:256], t2[:], AF.Ln,
                                         bias=sq_ap, scale=-2.0,
                                         accum_out=acc[:, COL_D[pi]:
                                                       COL_D[pi] + 1])
                    nc.scalar.activation(lb[:, 256:512], t[:, 256:512], AF.Ln,
                                         bias=sq_ap, scale=-2.0,
                                         accum_out=acc[:, COL_X[pi]:
                                                       COL_X[pi] + 1])
                    # same-label mask over diag (and corner when u=1)
                    mw = 384 if u == 1 else 256
                    lwin = labb[:, clo:clo + mw]
                    m = dscratch.tile([P, 384], BF16, tag="m")
                    nc.vector.tensor_scalar(m[:, 0:mw], lwin, lb_ap, None,
                                            ALU.is_equal)
                    prod = dscratch.tile([P, 384], BF16, tag="prod")
                    nc.vector.tensor_tensor(prod[:, 0:mw], m[:, 0:mw],
                                            lb[:, 0:mw], ALU.mult)
                    nc.vector.tensor_reduce(
                        acc[:, COL_M[pi]:COL_M[pi] + 1], prod[:, 0:256],
                        axis=mybir.AxisListType.X, op=ALU.add)
                    if u == 1:
                        nc.vector.tensor_reduce(
                            acc[:, COL_C[pi]:COL_C[pi] + 1],
                            prod[:, 256:384],
                            axis=mybir.AxisListType.X, op=ALU.add)

            nc.sync.dma_start(out=out_d[:], in_=acc[:])

    nc.compile()
    _PROG_CACHE["nc"] = nc
    return nc


def _host_prep(outputs, labels):
    """Sort rows by label, build per-core inputs + the host fp64 terms."""
    x = np.asarray(outputs, dtype=np.float32)
    lab = np.asarray(labels)
    assert x.shape == (N, D)
    perm = np.argsort(lab, kind="stable")
    xp = x[perm]
    labp = lab[perm].astype(np.float64)

    _, starts, counts = np.unique(labp, return_index=True, return_counts=True)
    assert counts.max() <= 128, f"label run {counts.max()} exceeds corner"

    xq8 = xp.astype(ml_dtypes.float8_e4m3)
    xq = xq8.astype(np.float64)
    # True (unquantized) norms make d2 = sq_i + sq_j - 2*xq_i.xq_j unbiased
    sq = (xp.astype(np.float64) ** 2).sum(axis=1)

    # host analytic terms (exact over quantized gram + exact norms)
    P_total = N * (N - 1) // 2
    P_same = float((counts * (counts - 1) // 2).sum())
    Sg_all = (np.dot(xq.sum(0), xq.sum(0)) - (xq * xq).sum()) / 2.0
    Sd2_all = (N - 1) * sq.sum() - 2.0 * Sg_all
    vs = np.add.reduceat(xq, starts, axis=0)
    qs_run = np.add.reduceat((xq * xq).sum(1), starts)
    sq_run = np.add.reduceat(sq, starts)
    Sg_same = ((vs * vs).sum(1) - qs_run).sum() / 2.0
    Sd2_same = ((counts - 1) * sq_run).sum() - 2.0 * Sg_same
    host_add = (C1 * LOG_B * P_total
                - C1 * (LOG_A + LOG_B) * P_same
                - 0.5 * B_C * Sd2_all
                + 0.5 * (A_C + B_C) * Sd2_same
                - 0.5 * C1 * N * LNE
                + C1 * N * LNE_BF)

    xt_q = np.ascontiguousarray(xq8.T)                             # [D, N]
    neg_half = -0.5 * sq
    hi = neg_half.astype(ml_dtypes.bfloat16)
    lo = (neg_half - hi.astype(np.float64)).astype(ml_dtypes.bfloat16)

    in_maps = []
    for d in range(NCORES):
        slabs = _core_slabs(d)
        cols = np.concatenate(
            [np.arange(b * BLK, (b + 1) * BLK) for b in slabs])
        # [P, KC, NB, BLK]: feature f = kc*128 + p
        xtp = np.ascontiguousarray(
            xt_q[:, cols].reshape(KC, P, NB, BLK).transpose(1, 0, 2, 3))
        aug = np.stack([hi[cols], lo[cols]])                       # [2, N]
        # label row for slot0(256) | slot1(256) | slot2 first 128
        lcols = np.concatenate([cols[0:512], cols[2 * BLK:2 * BLK + 128]])
        labrow = labp[lcols].astype(ml_dtypes.bfloat16)[None, :]   # [1, 640]

        rowd = np.zeros((P, 4 * 3), dtype=np.float64)
        for g, (slab, u) in enumerate(((0, 0), (0, 1), (1, 0), (1, 1))):
            rows = slabs[slab] * BLK + 128 * u + np.arange(P)
            sqr = sq[rows]
            rowd[:, 3 * g + 0] = sqr
            rowd[:, 3 * g + 1] = labp[rows]
            rowd[:, 3 * g + 2] = (sqr - EPS_D2) / 2.0
        im = {
            "aug": np.ascontiguousarray(aug),
            "lab": np.ascontiguousarray(labrow),
            "rowd": rowd.astype(np.float32),
        }
        for gi, (s0, gs) in enumerate(GROUPS):
            im[f"xg{gi}"] = np.ascontiguousarray(xtp[:, :, s0:s0 + gs, :])
        in_maps.append(im)
    return in_maps, host_add


def _combine(results, host_add):
    ncx = len(COL_X)
    s = np.zeros(4, dtype=np.float64)
    for r in results:
        o = np.asarray(r["out"], dtype=np.float64)
        s[0] += o[:, 0:ncx].sum()
        s[1] += o[:, list(COL_D.values())].sum()
        s[2] += o[:, list(COL_M.values())].sum()
        s[3] += o[:, list(COL_C.values())].sum()
    total = (C1 * s[0] + 0.5 * C1 * s[1] - C1 * s[2] - 2.0 * C1 * s[3]
             + host_add)
    return np.asarray(total, dtype=np.float32)


def kernel(**inputs):
    from concourse.bass_utils import run_bass_kernel_spmd
    nc = _build_program()
    in_maps, host_add = _host_prep(inputs["outputs"], inputs["labels"])
    res = run_bass_kernel_spmd(nc, in_maps, core_ids=list(range(NCORES)))
    return _combine(res.results, host_add)
